# revision 50
# baseline (speedup 1.0000x reference)
"""Trainium2 Bass kernel for nn_AdjGkNO (Galerkin Neural Operator).

Sharding: spatial axis S split 8 ways across NeuronCores (data for all 8
batches on every core). Per core the h state [B=8, C=64, SL=8192] lives in
SBUF as bf16 tiles in [(c + 64*b_parity), s] layout. Per layer:
  1. DMA-transpose h tiles into [s, (b, c)] form, accumulate the wbases
     projection x_co on TensorE (bf16, PSUM f32). Layer 0 shortcuts this:
     x_co(0) = fc0_ext^T (x^T wbases) is computed straight from the raw
     input (24-column matmul + VectorE outer products), so the first
     AllReduce issues ~10us in, fully overlapped with fc0.
  2. 128KB AllReduce of x_co across the 8 cores (internal DRAM bounce).
  3. Banded mode mixing as 94 small matmuls against a host-precomputed
     block-banded matrix; PE transposes re-layout x_co, small DMAs re-layout
     x_hat.
  4. x2 (pointwise conv, block-diag W) + x1 (bases projection) accumulate in
     the same PSUM tile; ScalarE applies bias+GELU; VectorE adds the residual
     into h. Last layer folds h into x2 via W+I and writes h directly.
  5. fc1 -> GELU -> fc2 tail streams per chunk.

Host side packs/shards all inputs (numpy only) and reassembles the output.
Self-contained: all shapes hardcoded.
"""

import numpy as np
import ml_dtypes

BF16 = ml_dtypes.bfloat16

NCORES = 8
B, S, IN_DIM, OUT_DIM = 8, 65536, 2, 1
C, M, DW, NL, FC = 64, 64, 3, 4, 128
SL = S // NCORES          # 8192 spatial positions per core

# mode-mix (out-chunk, in-chunk) pairs; chunk = 2 modes x 64 channels = 128
MIX_PAIRS = [(c, m) for c in range(32) for m in range(max(c - 1, 0), min(c + 1, 31) + 1)]
MIX_INDEX = {cm: i for i, cm in enumerate(MIX_PAIRS)}

# gelu(x) = 0.5x + s*P(s), s = x^2, for |x| <= 4; relu(x) outside.
# P = deg-6 least-squares fit of 0.5*erf(sqrt(s/2))/sqrt(s) on [0, 16].
GELU_H = [3.98732135e-01, -6.52466718e-02, 8.77588791e-03, -7.50773824e-04,
          3.52492348e-05, -6.79359824e-07]
GELU_DVE_EVERY = 10**9   # every 5th gelu chunk runs on DVE instead of ScalarE


def build(SL_=SL, no_ar=False, no_tr=False, skip_layers=False, nl_override=None):
    import concourse.bass as bass
    import concourse.bacc as bacc
    import concourse.mybir as mybir
    import concourse.tile as tile

    dt = mybir.dt
    f32, f32r, bf = dt.float32, dt.float32r, dt.bfloat16
    GELU = mybir.ActivationFunctionType.Gelu
    COPY = mybir.ActivationFunctionType.Copy
    ADD = mybir.AluOpType.add

    NT = SL_ // 128        # s-tiles per core
    NC2 = SL_ // 1024      # 1024-wide chunks per j-block

    nc = bacc.Bacc("TRN2", target_bir_lowering=False, debug=False,
                   num_devices=NCORES)

    xs_d = nc.dram_tensor("xs", [5, 4, SL_], bf, kind="ExternalInput")
    fc0_d = nc.dram_tensor("fc0bd", [5, 128], bf, kind="ExternalInput")
    wbt_d = nc.dram_tensor("wbt", [128, NT * 64], bf, kind="ExternalInput")
    bast_d = nc.dram_tensor("bast", [128, SL_ // 2], bf, kind="ExternalInput")
    tp_d = nc.dram_tensor("tp", [NL, 128, 94 * 128], bf, kind="ExternalInput")
    wbd_d = nc.dram_tensor("wbd", [128, NL * 128], bf, kind="ExternalInput")
    biasv_d = nc.dram_tensor("biasv", [128, NL], f32, kind="ExternalInput")
    fc1w_d = nc.dram_tensor("fc1w", [128, 128], bf, kind="ExternalInput")
    fc1b_d = nc.dram_tensor("fc1b", [128, 1], f32, kind="ExternalInput")
    fc2w_d = nc.dram_tensor("fc2w", [128, 1], bf, kind="ExternalInput")
    id8_d = nc.dram_tensor("id8", [8, 8], bf, kind="ExternalInput")
    xq_d = nc.dram_tensor("xq", [128, (SL_ // 128) * 24], bf, kind="ExternalInput")
    fc0r_d = nc.dram_tensor("fc0rep", [64, 192], bf, kind="ExternalInput")
    out_d = nc.dram_tensor("out", [B, 128, NT], f32, kind="ExternalOutput")

    from contextlib import ExitStack
    with tile.TileContext(nc) as tc:
        with ExitStack() as ctx:
            ep = ctx.enter_context
            cpool = ep(tc.tile_pool(name="const", bufs=1))
            hpool = ep(tc.tile_pool(name="hcs", bufs=1))
            qtpool = ep(tc.tile_pool(name="qt", bufs=3))
            stgpool = ep(tc.tile_pool(name="stg", bufs=1))
            arsbpool = ep(tc.tile_pool(name="arsb", bufs=1))
            xcospool = ep(tc.tile_pool(name="xcos", bufs=2))
            xhatpool = ep(tc.tile_pool(name="xhat", bufs=1))
            xhtpool = ep(tc.tile_pool(name="xht", bufs=2))
            respool = ep(tc.tile_pool(name="res", bufs=3))
            gpool = ep(tc.tile_pool(name="gbuf", bufs=3))
            obpool = ep(tc.tile_pool(name="outb", bufs=2))
            xspool = ep(tc.tile_pool(name="xsb", bufs=2))
            tppool = ep(tc.tile_pool(name="tpp", bufs=1))
            gtmpp = ep(tc.tile_pool(name="gtmp", bufs=1))
            pbig = ep(tc.tile_pool(name="pbig", bufs=3, space="PSUM"))
            ptp = ep(tc.tile_pool(name="pt", bufs=2, space="PSUM"))
            dram = ep(tc.tile_pool(name="dram", bufs=2, space="DRAM"))
            # ---- constants ----
            fc0w = cpool.tile([5, 128], bf, tag="fc0w")
            nc.sync.dma_start(fc0w[:], fc0_d[:])
            wbt = cpool.tile([128, NT * 64], bf, tag="wbt")
            nc.sync.dma_start(wbt[:], wbt_d[:])
            bast0 = cpool.tile([64, SL_ // 2], bf, tag="bast0")
            nc.sync.dma_start(bast0[:], bast_d[0:64])
            bast1 = cpool.tile([64, SL_ // 2], bf, tag="bast1")
            nc.sync.dma_start(bast1[:], bast_d[64:128])
            wbd = cpool.tile([128, NL * 128], bf, tag="wbd")
            nc.sync.dma_start(wbd[:], wbd_d[:])
            biasv = cpool.tile([128, NL], f32, tag="biasv")
            nc.sync.dma_start(biasv[:], biasv_d[:])
            fc1w = cpool.tile([128, 128], bf, tag="fc1w")
            nc.sync.dma_start(fc1w[:], fc1w_d[:])
            fc1b = cpool.tile([128, 1], f32, tag="fc1b")
            nc.sync.dma_start(fc1b[:], fc1b_d[:])
            fc2w = cpool.tile([128, 1], bf, tag="fc2w")
            nc.sync.dma_start(fc2w[:], fc2w_d[:])
            id8 = cpool.tile([8, 8], bf, tag="id8")
            nc.sync.dma_start(id8[:], id8_d[:])
            xq = cpool.tile([128, NT * 24], bf, tag="xq")
            nc.sync.dma_start(xq[:], xq_d[:])
            fc0r = cpool.tile([64, 192], bf, tag="fc0r")
            nc.sync.dma_start(fc0r[:], fc0r_d[:])

            NCH = max(SL_ // 2048, 1)
            CW = SL_ // NCH                      # tile width (2048 normally)
            hcs_t = [[hpool.tile([128, CW], bf, tag=f"hcs{j}_{c}",
                                 name=f"hcs{j}_{c}")
                      for c in range(NCH)] for j in range(4)]

            def hsl(j, c2, lo, hi):
                
                t = hcs_t[j][(c2 * 1024 + lo) // CW]
                off = (c2 * 1024 + lo) % CW
                return t[:, off:off + (hi - lo)]

            def hslp(j, c2, pb, lo, hi):
                t = hcs_t[j][(c2 * 1024 + lo) // CW]
                off = (c2 * 1024 + lo) % CW
                return t[pb * 64:(pb + 1) * 64, off:off + (hi - lo)]

            MULT = mybir.AluOpType.mult
            GE = mybir.AluOpType.is_ge

            def dve_gelu(out_ap, pm_ap, bias_ap, tmp):
                # out = gelu(pm + bias) computed on VectorE (bf16 4x ops)
                x, s_, u, msk = tmp
                nc.vector.tensor_scalar_add(x[:], pm_ap, bias_ap)
                nc.vector.tensor_tensor(s_[:], x[:], x[:], MULT)
                nc.vector.tensor_scalar(u[:], s_[:], GELU_H[5], GELU_H[4],
                                        MULT, mybir.AluOpType.add)
                for k in (3, 2, 1, 0):
                    nc.vector.tensor_tensor(u[:], u[:], s_[:], MULT)
                    nc.vector.tensor_scalar_add(u[:], u[:], GELU_H[k])
                # out = 0.5*x + u*s  (= 0.5x + H(s))
                nc.vector.tensor_tensor(u[:], u[:], s_[:], MULT)
                nc.vector.scalar_tensor_tensor(
                    out_ap, x[:], 0.5, u[:], op0=MULT, op1=mybir.AluOpType.add)
                # tail: s > 16 -> relu(x)
                nc.vector.tensor_scalar(msk[:], s_[:], 16.0, None, GE)
                nc.vector.tensor_scalar_max(u[:], x[:], 0.0)
                nc.vector.copy_predicated(out_ap, msk[:], u[:])

            # ---- fc0 ----
            XW = min(2048, SL_)
            for j in range(4):
                for cx in range(SL_ // XW):
                    xst = xspool.tile([5, XW], bf, tag="xs")
                    nc.sync.dma_start(xst[:], xs_d[:, j, cx * XW:(cx + 1) * XW])
                    for ci in range(XW // 1024):
                        c2 = cx * (XW // 1024) + ci
                        pm = pbig.tile([128, 1024], f32, tag="pbig")
                        for hh in range(2):
                            nc.tensor.matmul(
                                pm[:, hh * 512:(hh + 1) * 512],
                                fc0w[:],
                                xst[:, ci * 1024 + hh * 512:ci * 1024 + (hh + 1) * 512],
                                start=True, stop=True)
                        if c2 % 4 == 3:
                            nc.scalar.activation(hsl(j, c2, 0, 1024), pm[:], COPY)
                        else:
                            nc.vector.tensor_copy(hsl(j, c2, 0, 1024), pm[:])

            # ---- layers ----
            for l in ([] if skip_layers else range(nl_override if nl_override is not None else NL)):
                tp = tppool.tile([128, 94 * 128], bf, tag="tp")
                for tq in range(4):
                    lo = tq * 24 * 128
                    hi = min((tq + 1) * 24 * 128, 94 * 128)
                    nc.gpsimd.dma_start(tp[:, lo:hi], tp_d[l][:, lo:hi])

                # 1) x_co. Layer 0: algebraic shortcut straight from x
                # (x_co = fc0^T (x^T wb)), so the AllReduce fires without
                # waiting for fc0/h/transposes. Other layers: DMA-transpose
                # h and accumulate.
                if l == 0:
                    pxw = ptp.tile([64, 24], f32, tag="pt", name="pxw")
                    for t in range(NT):
                        nc.tensor.matmul(
                            pxw[:], wbt[:, t * 64:(t + 1) * 64],
                            xq[:, t * 24:(t + 1) * 24],
                            start=(t == 0), stop=(t == NT - 1))
                    xwsb = stgpool.tile([64, 24], f32, tag="xwsb")
                    nc.vector.tensor_copy(xwsb[:], pxw[:])
                    xc0 = stgpool.tile([64, 512], bf, tag="stg")
                    for b in range(8):
                        for dd in range(3):
                            r = b * 3 + dd
                            if dd == 0:
                                nc.vector.tensor_scalar(
                                    xc0[:, b * 64:(b + 1) * 64],
                                    fc0r[:, dd * 64:(dd + 1) * 64],
                                    xwsb[:, r:r + 1], None, MULT)
                            else:
                                nc.vector.scalar_tensor_tensor(
                                    xc0[:, b * 64:(b + 1) * 64],
                                    fc0r[:, dd * 64:(dd + 1) * 64],
                                    xwsb[:, r:r + 1],
                                    xc0[:, b * 64:(b + 1) * 64],
                                    op0=MULT, op1=ADD)
                    arin = dram.tile([8, 4096], bf, tag="arin")
                    arout = dram.tile([8, 4096], bf, tag="arout",
                                      addr_space="Shared")
                    nc.sync.dma_start(
                        arin[:].rearrange("b (k i) -> k b i", i=64),
                        xc0[:].rearrange("k (b i) -> k b i", b=8))
                else:
                    pco = ptp.tile([64, 512], f32, tag="pt", name=f"pco_{l}")
                NTW = CW // 128                  # s-tiles per hcs tile
                for ch in ([] if l == 0 else range(NCH)):
                    qt = qtpool.tile([128, NTW * 512], bf, tag="qt")
                    qt3 = qt[:].rearrange("p (t x) -> p t x", t=NTW)
                    for j in range(4):
                        for pb in range(2):
                            off = (j // 2) * 256 + ((j % 2) * 2 + pb) * 64
                            nc.sync.dma_start_transpose(
                                qt3[:, :, off:off + 64],
                                hcs_t[j][ch][pb * 64:(pb + 1) * 64, :])
                    for tt in range(NTW):
                        t = ch * NTW + tt
                        nc.tensor.matmul(
                            pco[:],
                            wbt[:, t * 64:(t + 1) * 64],
                            qt[:, tt * 512:(tt + 1) * 512],
                            start=(t == 0), stop=(t == NT - 1))

                # 2) stage x_co, AllReduce
                if l > 0:
                    stg = stgpool.tile([64, 512], bf, tag="stg")
                    nc.vector.tensor_copy(stg[:], pco[:])
                    arin = dram.tile([8, 4096], bf, tag="arin")
                    arout = dram.tile([8, 4096], bf, tag="arout",
                                      addr_space="Shared")
                    for g in range(2):
                        nc.sync.dma_start(
                            arin[g * 4:(g + 1) * 4]
                            .rearrange("b (k i) -> k b i", i=64),
                            stg[:, g * 256:(g + 1) * 256]
                            .rearrange("k (b i) -> k b i", b=4))
                if no_ar:
                    nc.sync.dma_start(arout[:], arin[:])
                else:
                    nc.gpsimd.collective_compute(
                        "AllReduce", ADD,
                        replica_groups=[list(range(NCORES))],
                        ins=[arin[:].opt()], outs=[arout[:].opt()])
                arsb = arsbpool.tile([8, 4096], bf, tag="arsb")
                nc.sync.dma_start(arsb[:], arout[:])

                # 3) mode mixing
                ptr = ptp.tile([128, 256], bf, tag="pt")
                for m in range(32):
                    nc.tensor.transpose(
                        ptr[:, m * 8:(m + 1) * 8],
                        arsb[:, m * 128:(m + 1) * 128], id8[:])
                xcos = xcospool.tile([128, 256], bf, tag="xcos")
                nc.vector.tensor_copy(xcos[:], ptr[:])

                xhat = xhatpool.tile([8, 4096], bf, tag="xhat")
                xhd = dram.tile([8, 4096], bf, tag="xhd")
                for grp in range(8):
                    pm8 = ptp.tile([8, 512], f32, tag="pt", name=f"pm8_{l}_{grp}")
                    for cc in range(4):
                        c = grp * 4 + cc
                        nbrs = list(range(max(c - 1, 0), min(c + 1, 31) + 1))
                        for mi, m in enumerate(nbrs):
                            ci = MIX_INDEX[(c, m)]
                            nc.tensor.matmul(
                                pm8[:, cc * 128:(cc + 1) * 128],
                                xcos[:, m * 8:(m + 1) * 8],
                                tp[:, ci * 128:(ci + 1) * 128],
                                start=(mi == 0), stop=(mi == len(nbrs) - 1))
                    nc.vector.tensor_copy(
                        xhat[:, grp * 512:(grp + 1) * 512], pm8[:])

                nc.sync.dma_start(xhd[:], xhat[:])
                xht = [xhtpool.tile([64, 128], bf, tag=f"xht{j}",
                                    name=f"xht{j}_{l}") for j in range(4)]
                for j in range(4):
                    nc.sync.dma_start(
                        xht[j][:].rearrange("k (pb o) -> k pb o", pb=2),
                        xhd[2 * j:2 * j + 2].rearrange("pb (k o) -> k pb o", o=64))

                # 4) x2 + x1 -> bias (+gelu) -> residual
                for j in range(4):
                    for c2 in range(NC2):
                        pm = pbig.tile([128, 1024], f32, tag="pbig")
                        for hh in range(2):
                            sc = c2 * 2 + hh
                            hso = sc // (NC2)       # half-of-SL selector
                            soff = (sc % NC2) * 512
                            nc.tensor.matmul(
                                pm[:, hh * 512:(hh + 1) * 512],
                                wbd[:, l * 128:(l + 1) * 128],
                                hsl(j, c2, hh * 512, (hh + 1) * 512),
                                start=True, stop=False)
                            nc.tensor.matmul(
                                pm[:, hh * 512:(hh + 1) * 512],
                                xht[j][:],
                                (bast0 if hso == 0 else bast1)[:, soff:soff + 512],
                                start=False, stop=True)
                        if l != NL - 1:
                            res = respool.tile([128, 1024], bf, tag="res")
                            gcnt = l * 32 + j * NC2 + c2
                            if gcnt % GELU_DVE_EVERY == GELU_DVE_EVERY - 1:
                                tmp = [gp.tile([128, 1024], bf, tag=f"gtmp{i}",
                                               name=f"gtmp{i}_{gcnt}")
                                       for i, gp in enumerate(
                                           [gtmpp, gtmpp, gtmpp, gtmpp])]
                                dve_gelu(res[:], pm[:], biasv[:, l:l + 1], tmp)
                            else:
                                nc.scalar.activation(res[:], pm[:], GELU,
                                                     bias=biasv[:, l:l + 1])
                            nc.vector.tensor_tensor(
                                hsl(j, c2, 0, 1024), hsl(j, c2, 0, 1024),
                                res[:], ADD)
                        else:
                            nc.vector.tensor_scalar_add(
                                hsl(j, c2, 0, 1024), pm[:], biasv[:, l:l + 1])

            # ---- tail: fc1 -> gelu -> fc2 ----
            for j in range(4):
                for pb in range(2):
                    b = 2 * j + pb
                    po = ptp.tile([128, NT], f32, tag="pt")
                    for c2 in range(NC2):
                        pf = pbig.tile([128, 1024], f32, tag="pbig")
                        for hh in range(2):
                            nc.tensor.matmul(
                                pf[:, hh * 512:(hh + 1) * 512],
                                fc1w[pb * 64:(pb + 1) * 64, :],
                                hslp(j, c2, pb, hh * 512, (hh + 1) * 512),
                                start=True, stop=True)
                        g = gpool.tile([128, 1024], bf, tag="g")
                        tcnt = (j * 2 + pb) * NC2 + c2
                        if tcnt % GELU_DVE_EVERY == GELU_DVE_EVERY - 1:
                            tmp = [gp.tile([128, 1024], bf, tag=f"gtmp{i}",
                                           name=f"gtmpt{i}_{tcnt}")
                                   for i, gp in enumerate(
                                       [gtmpp, gtmpp, gtmpp, gtmpp])]
                            dve_gelu(g[:], pf[:], fc1b[:], tmp)
                        else:
                            nc.scalar.activation(g[:], pf[:], GELU, bias=fc1b[:])
                        for tt in range(8):
                            nc.tensor.matmul(
                                po[:, c2 * 8 + tt:c2 * 8 + tt + 1],
                                g[:, tt * 128:(tt + 1) * 128],
                                fc2w[:], start=True, stop=True)
                    ob = obpool.tile([128, NT], f32, tag="ob")
                    nc.vector.tensor_copy(ob[:], po[:])
                    nc.gpsimd.dma_start(out_d[b], ob[:])

    nc.compile()
    return nc


def _build_T(wlist):
    """Dense block-banded mode-mixing matrix T[k*C+o, k2*C+i] (f32)."""
    T = np.zeros((M * C, M * C), np.float32)
    w0 = np.asarray(wlist[0], np.float32)
    for k in range(M):
        T[k * C:(k + 1) * C, k * C:(k + 1) * C] = w0[:, :, k].T
    for d in (1, 2):
        w_up = np.asarray(wlist[2 * d - 1], np.float32)
        w_dn = np.asarray(wlist[2 * d], np.float32)
        for k in range(M - d):
            T[k * C:(k + 1) * C, (k + d) * C:(k + d + 1) * C] = w_up[:, :, k].T
        for k in range(d, M):
            T[k * C:(k + 1) * C, (k - d) * C:(k - d + 1) * C] = w_dn[:, :, k - d].T
    return T


def prep_inputs(x, bases, wbases, sp_weights, w_weights, w_biases,
                fc0_w, fc0_b, fc1_w, fc1_b, fc2_w, fc2_b, SL_=SL, ncores=NCORES):
    """Host-side packing. Returns (in_maps list, fc2_b scalar)."""
    x = np.asarray(x, np.float32)
    bases = np.asarray(bases, np.float32)
    wbases = np.asarray(wbases, np.float32)
    NT = SL_ // 128

    # shared tensors
    fc0bd_f = np.zeros((5, 128), np.float32)
    fw = np.asarray(fc0_w, np.float32)
    fb = np.asarray(fc0_b, np.float32)
    for p2 in range(2):
        for i in range(IN_DIM):
            fc0bd_f[i + 2 * p2, p2 * 64:(p2 + 1) * 64] = fw[i]
        fc0bd_f[4, p2 * 64:(p2 + 1) * 64] = fb
    fc0bd = fc0bd_f.astype(BF16)

    tp = np.zeros((NL, 128, 94 * 128), BF16)
    for l in range(NL):
        T = _build_T(sp_weights[l])
        for ci, (c, m) in enumerate(MIX_PAIRS):
            blk = T[2 * c * C:(2 * c + 2) * C, m * 128:(m + 1) * 128]  # [128out,128in]
            tp[l, :, ci * 128:(ci + 1) * 128] = blk.T.astype(BF16)

    wbd = np.zeros((128, NL * 128), np.float32)
    for l in range(NL):
        W = np.asarray(w_weights[l], np.float32)
        if l == NL - 1:
            W = W + np.eye(C, dtype=np.float32)
        for p in range(2):
            wbd[p * 64:(p + 1) * 64, l * 128 + p * 64:l * 128 + (p + 1) * 64] = W
    wbd = wbd.astype(BF16)

    biasv = np.zeros((128, NL), np.float32)
    for l in range(NL):
        bl = np.asarray(w_biases[l], np.float32)
        biasv[:64, l] = bl
        biasv[64:, l] = bl

    fc1w = np.tile(np.asarray(fc1_w, np.float32), (2, 1)).astype(BF16)  # [128,128]
    fc0rep = np.zeros((64, 192), np.float32)
    for dd in range(3):
        fc0rep[:, dd * 64:(dd + 1) * 64] = (fw[dd] if dd < IN_DIM else fb)[None, :]
    fc0rep = fc0rep.astype(BF16)
    fc1b = np.asarray(fc1_b, np.float32).reshape(128, 1)
    fc2w = np.asarray(fc2_w, np.float32).reshape(128, 1).astype(BF16)
    id8 = np.eye(8, dtype=np.float32).astype(BF16)

    in_maps = []
    for r in range(ncores):
        sl = slice(r * SL_, (r + 1) * SL_)
        xr = x[:, sl, :]                                        # [8, SL, 2]
        xs_f = np.ones((5, 4, SL_), np.float32)
        for jj in range(4):
            for pb in range(2):
                for i in range(IN_DIM):
                    xs_f[i + 2 * pb, jj] = xr[2 * jj + pb, :, i]
        xs = xs_f.astype(BF16)

        wbr = wbases[sl]                                        # [SL, 64]
        wbt = np.zeros((128, NT * 64), np.float32)
        for t in range(NT):
            wbt[:, t * 64:(t + 1) * 64] = wbr[t * 128:(t + 1) * 128]
        wbt = wbt.astype(BF16)

        bar = bases[sl]                                         # [SL, 64]
        bast = np.zeros((128, SL_ // 2), np.float32)
        half = SL_ // 2
        bast[:64, :] = bar[:half].T
        bast[64:, :] = bar[half:].T
        bast = bast.astype(BF16)

        NTr = SL_ // 128
        xq = np.ones((128, NTr, 8, 3), np.float32)
        for dd in range(IN_DIM):
            xq[:, :, :, dd] = xr[:, :, dd].T.reshape(NTr, 128, 8).transpose(1, 0, 2)
        xq = xq.reshape(128, NTr * 24).astype(BF16)

        in_maps.append(dict(
            xs=xs, fc0bd=fc0bd, wbt=wbt, bast=bast, tp=tp, wbd=wbd,
            biasv=biasv, fc1w=fc1w, fc1b=fc1b, fc2w=fc2w,
            id8=id8, xq=xq, fc0rep=fc0rep))
    return in_maps


_CACHE = {}


def kernel(**inputs):
    from concourse.bass_utils import run_bass_kernel_spmd

    if "nc" not in _CACHE:
        _CACHE["nc"] = build(SL)
    nc = _CACHE["nc"]

    in_maps = prep_inputs(**inputs)
    res = run_bass_kernel_spmd(nc, in_maps, core_ids=list(range(NCORES)))

    out = np.zeros((B, S, OUT_DIM), np.float32)
    NT = SL // 128
    fc2_b = np.asarray(inputs["fc2_b"], np.float32)
    for r in range(NCORES):
        o = res.results[r]["out"]                # [B, 128, NT]
        # s_local = t*128 + p
        out[:, r * SL:(r + 1) * SL, 0] = o.transpose(0, 2, 1).reshape(B, SL)
    out += fc2_b[0]
    return out
